# revision 53
# baseline (speedup 1.0000x reference)
"""Causal self-attention (12 heads, T=1024, C=768, prefix P=4) on 8 TRN2 cores.

Sharding: data-parallel over batch B=8 -> one batch element per NeuronCore.
No collectives. Weights are replicated to every core.

Per-core kernel (fp16 operands, fp32 psum accumulation; fp8 was tried
and rejected -- e4m3 quantization of any of x/wqk/wv alone costs >=1e-2
rel err vs the 2e-2 gate):
  qkv projection from chunk-major tiles: qT pair tile [128, T] (head 2p
    rows 0:64, head 2p+1 rows 64:128), kh pair tile [128, T] (same row
    split), v [128, H, 128] per t-chunk (col 64 = 1.0 -> softmax denom).
  scores (r, window c): the two heads' K=64 matmuls are ROW-TILED
    (tile_position (0,0)/(64,0)) so they run concurrently on the two
    half-arrays, writing the two banks of one 2-bank psum tile; ONE
    merged exp covers both heads ((N+352)/1.2ns amortization). Diagonal
    band tiles get one broadcast triangular-mask multiply; fully-masked
    columns are never computed.
  prefix scores: two row-tiled K=64 matmuls (out rows 0:4 / 32:36) ->
    pp -> two [4, W] exps (only written rows -- reading unwritten psum
    rows is a race); the row-tiled K=4 AV-prefix matmuls OPEN each py
    accumulation group since their exp is ready first.
  AV: py[0:65, t] = [y(64 dims); denom] accumulated over kv chunks.
  norm: DVE copies py -> sb; the two denom rows bounce via DRAM into a
    [64, 16] layout for a lane-parallel reciprocal (DVE reciprocal is
    ~8 cyc/elem/lane -- a [2, W] row recip costs 3.3us, lane-parallel
    costs 0.24us), return as an f16 [2, W] row pair, ONE K=2 f16 matmul
    broadcasts both heads' 1/D, DVE multiplies into the f16 yT tiles.
  out = yT.T @ w_proj + b_proj -> [T, 768] f16 DMA out.

Schedule: merged exp per unit still outpaces the row-tiled matmul pair,
so every scores window interleaves scalar-independent PE fillers between
its units: qkproj(p+1) groups in window (p,1), av(p,1) chunks in window
(p+1,0), and pairs-0..4 output-projection partials (spilled +bias to
SBUF f32) in the last pair's windows. norm_mul trails its norm_pre by a
full window so the ones-matmul never waits on the denominator roundtrip;
the tail is just pair-5 outproj matmuls + fused spill adds.
"""

import numpy as np
from contextlib import ExitStack

import concourse.bass as bass
import concourse.mybir as mybir
import concourse.tile as tile
from concourse import bacc
from concourse.bass_utils import run_bass_kernel_spmd

F32 = mybir.dt.float32
F32R = mybir.dt.float32r
F16 = mybir.dt.float16
N_CORES = 8
T, C, H, D, PFX = 1024, 768, 12, 64, 4
NPAIR = H // 2          # 6 head pairs
KC = C // 128           # 6 contraction chunks
W = 512                 # T window for scores
NW = T // W             # 2 windows
TCH = T // 128          # 8 T chunks
EXP = mybir.ActivationFunctionType.Exp
COPY = mybir.ActivationFunctionType.Copy
F8 = mybir.dt.float8e4
DR = mybir.MatmulPerfMode.DoubleRow
KC2 = KC // 2            # 3 double-chunks of 256 for fp8 DoubleRow
WS = 1.0                 # (fp8 path removed: quantization error ~1.4e-2)
SCALE = 1.0 / np.sqrt(D)
SCALE_E = SCALE / (WS * WS)   # exp scale absorbs q'=WS*q, k'=WS*k


def _build():
    nc = bacc.Bacc("TRN2", target_bir_lowering=False, debug=False,
                   num_devices=N_CORES)
    xt_d = nc.declare_dram_parameter("xt", [128, KC, T], F16, isOutput=False)
    wq_d = [nc.declare_dram_parameter(f"wq{p}", [128, KC, 2, 128], F16,
                                      isOutput=False) for p in range(NPAIR)]
    wv_d = nc.declare_dram_parameter("wv", [128, KC, C], F16, isOutput=False)
    wp_d = nc.declare_dram_parameter("wp", [128, KC, C], F16, isOutput=False)
    bqk_d = nc.declare_dram_parameter("b_qk", [128, 12], F32, isOutput=False)
    bv_d = nc.declare_dram_parameter("bv_bc", [128, C], F32, isOutput=False)
    bp_d = nc.declare_dram_parameter("bp_bc", [128, C], F32, isOutput=False)
    ktc_d = nc.declare_dram_parameter("kTc4", [128, NPAIR, 4], F16,
                                      isOutput=False)
    vcp_d = nc.declare_dram_parameter("vcP", [64, H, 128], F16, isOutput=False)
    tri_d = nc.declare_dram_parameter("tri", [128, 128], F16, isOutput=False)
    ones_d = nc.declare_dram_parameter("ones2", [2, 128], F16, isOutput=False)
    out_d = nc.declare_dram_parameter("out", [T, C], F16, isOutput=True)

    with tile.TileContext(nc) as tc, ExitStack() as ctx:
        pers = ctx.enter_context(tc.tile_pool(name="pers", bufs=1))
        wqp = ctx.enter_context(tc.tile_pool(name="wqp", bufs=6))
        qkp = ctx.enter_context(tc.tile_pool(name="qkp", bufs=2))
        khp = ctx.enter_context(tc.tile_pool(name="khp", bufs=2))
        ep = ctx.enter_context(tc.tile_pool(name="ep", bufs=13))
        epp = ctx.enter_context(tc.tile_pool(name="epp", bufs=6))
        sbp = ctx.enter_context(tc.tile_pool(name="sbp", bufs=6))
        rwp = ctx.enter_context(tc.tile_pool(name="rwp", bufs=2))
        drp = ctx.enter_context(tc.tile_pool(name="drp", bufs=2))
        dram = ctx.enter_context(tc.tile_pool(name="dram", bufs=4,
                                              space="DRAM"))
        op = ctx.enter_context(tc.tile_pool(name="op", bufs=4))
        ps = ctx.enter_context(tc.tile_pool(name="ps", bufs=2, space="PSUM"))
        ps2 = ctx.enter_context(tc.tile_pool(name="ps2", bufs=2, space="PSUM"))
        pyp = ctx.enter_context(tc.tile_pool(name="pyp", bufs=2, space="PSUM"))

        # ---- PE warmup: HAM needs ~3.4us of activity to unthrottle ------
        wtile = pers.tile([128, W], F16, tag="wtile")
        nc.vector.memset(wtile[:], 0.0)
        pwarm = ps.tile([128, 512], F32, tag="ps", name="pwarm")
        for i in range(12):
            nc.tensor.matmul(pwarm[:], wtile[:, 0:128], wtile[:],
                             start=True, stop=True)

        # ---- persistent loads, priority order ---------------------------
        wq = [None] * NPAIR
        wq[0] = wqp.tile([128, KC, 2, 128], F16, tag="wq", name="wq0")
        nc.sync.dma_start(wq[0][:], wq_d[0][:])
        xt = pers.tile([128, KC, T], F16, tag="xt")
        for k in range(KC):  # per-chunk so the first qkproj matmuls start
            nc.sync.dma_start(xt[:, k, :], xt_d[:, k, :])
        bqk = pers.tile([128, 12], F32, tag="bqk")
        nc.sync.dma_start(bqk[:], bqk_d[:])
        wv = pers.tile([128, KC, C], F16, tag="wv")
        nc.sync.dma_start(wv[:], wv_d[:])
        bv = pers.tile([128, C], F32, tag="bv")
        nc.sync.dma_start(bv[:], bv_d[:])
        tri = pers.tile([128, 128], F16, tag="tri")
        nc.sync.dma_start(tri[:], tri_d[:])
        ktc = pers.tile([128, NPAIR, 4], F16, tag="ktc")
        nc.sync.dma_start(ktc[:], ktc_d[:])
        vcp = pers.tile([64, H, 128], F16, tag="vcp")
        nc.sync.dma_start(vcp[:], vcp_d[:])
        for p in range(1, NPAIR):
            wq[p] = wqp.tile([128, KC, 2, 128], F16, tag="wq",
                             name=f"wq{p}")
            nc.sync.dma_start(wq[p][:], wq_d[p][:])

        wp = pers.tile([128, KC, C], F16, tag="wp")
        nc.sync.dma_start(wp[:], wp_d[:])
        bp = pers.tile([128, C], F32, tag="bp")
        nc.sync.dma_start(bp[:], bp_d[:])

        ones1 = pers.tile([66, 128], F16, tag="ones1")
        nc.sync.dma_start(ones1[64:66, :], ones_d[:])

        yT = [pers.tile([128, T], F16, tag=f"yT{p}", name=f"yT{p}")
              for p in range(NPAIR)]

        # ---- phases ----
        qk_tiles = {}
        ets = {}
        etps = {}
        pys = {}

        def qkproj(p, groups=((0, 0), (0, 1), (1, 0), (1, 1))):
            if p in qk_tiles:
                qT, kh = qk_tiles[p]
            else:
                qT = qkp.tile([128, T], F16, tag="qT", name=f"qT{p}")
                # pair tile: head 2p's k-features at rows 0:64, head 2p+1's
                # at rows 64:128 (aligned with the q rows in qT).
                kh = khp.tile([128, T], F16, tag="kh", name=f"kh{p}")
                qk_tiles[p] = (qT, kh)
            for half, w in groups:
                pq = ps.tile([128, 512], F32, tag="ps",
                             name=f"pq{p}_{half}_{w}")
                for k in range(KC):
                    nc.tensor.matmul(pq[:], wq[p][:, k, half, :],
                                     xt[:, k, W * w:W * w + W],
                                     start=(k == 0), stop=(k == KC - 1))
                if half == 0:
                    nc.vector.tensor_scalar_add(
                        qT[:, W * w:W * w + W], pq[:], bqk[:, p:p + 1])
                else:
                    nc.vector.tensor_scalar_add(
                        kh[:, W * w:W * w + W], pq[:], bqk[:, 6 + p:7 + p])

        vt = [None] * TCH

        def vproj(mt):
            v_ = pers.tile([128, H, 128], F16, tag=f"v{mt}")
            nc.gpsimd.memset(v_[:, :, 64:65], 1.0)
            nc.gpsimd.memset(v_[:, :, 65:128], 0.0)
            for n0, nsz in ((0, 512), (512, 256)):
                pv = ps.tile([128, 512], F32, tag="ps", name=f"pv{mt}_{n0}")
                for k in range(KC):
                    nc.tensor.matmul(pv[:, :nsz],
                                     xt[:, k, 128 * mt:128 * mt + 128],
                                     wv[:, k, n0:n0 + nsz],
                                     start=(k == 0), stop=(k == KC - 1))
                h0, hn = n0 // 64, nsz // 64
                nc.vector.tensor_add(
                    v_[:, h0:h0 + hn, 0:64],
                    pv[:, :nsz].rearrange("a (h d) -> a h d", d=64),
                    bv[:, n0:n0 + nsz].rearrange("a (h d) -> a h d", d=64))
            vt[mt] = v_

        def scores_w(p, c, fillers=()):
            """Window c of pair p. The two heads' K=64 score matmuls are
            row-tiled (tile_position (0,0)/(64,0)) -> they run CONCURRENTLY
            on the upper/lower halves of the PE array, writing the two
            banks of one 2-bank psum tile; ONE merged exp covers both
            heads ((N+352)/1.2 scalar cost amortizes the 352-cy overhead).
            `fillers` are scalar-independent emission thunks (qkproj
            groups / vproj chunks) interleaved between units so the PE
            never starves while the exp queue drains."""
            fillers = list(fillers)
            qT, kh = qk_tiles[p]
            # prefix first: the two heads' K=64 prefix matmuls are also
            # row-tiled (out rows 0:4 and 32:36); their exp lands early
            # in the scalar queue so the AV-prefix matmuls never stall.
            pp = ps.tile([128, 512], F32, tag="ps", name=f"pp{p}_{c}")
            for s in range(2):
                nc.tensor.matmul(pp[32 * s:32 * s + 4, :],
                                 ktc[64 * s:64 * s + 64, p, :],
                                 qT[64 * s:64 * s + 64,
                                    W * c:W * (c + 1)],
                                 start=True, stop=True,
                                 tile_position=(64 * s, 32 * s))
            ep_ = epp.tile([36, W], F16, tag="etp", name=f"etp{p}_{c}")
            for s in range(2):  # only rows 32s:32s+4 were written
                nc.scalar.activation(ep_[32 * s:32 * s + 4, :],
                                     pp[32 * s:32 * s + 4, :], EXP,
                                     scale=float(SCALE_E))
            etps[(p, c)] = ep_
            nun = 4 * c + 4
            for r in range(nun):
                # both heads' e for (c, r) share one [128, 2, W] tile
                e2 = ep.tile([128, 2, W], F16, tag="et",
                             name=f"et{p}_{c}_{r}")
                ets[(p, c, r)] = e2
                j0 = 128 * r - W * c if r >= 4 * c else 0
                pt = ps2.tile([128, 2, 512], F32, tag="ps2",
                              name=f"pss{p}_{c}_{r}")
                for s in range(2):
                    nc.tensor.matmul(
                        pt[:, s, j0:W],
                        kh[64 * s:64 * s + 64, 128 * r:128 * r + 128],
                        qT[64 * s:64 * s + 64, W * c + j0:W * (c + 1)],
                        start=True, stop=True,
                        tile_position=(64 * s, 0))
                nc.scalar.activation(e2[:, :, j0:W], pt[:, :, j0:W],
                                     EXP, scale=float(SCALE_E))
                if r >= 4 * c:  # one masked multiply covers both heads
                    nc.vector.tensor_mul(
                        e2[:, :, j0:j0 + 128], e2[:, :, j0:j0 + 128],
                        tri[:].unsqueeze(1).broadcast_to((128, 2, 128)))
                # spread fillers across the remaining units
                nf = len(fillers) * (r + 1) // nun
                while fillers and nf:
                    fillers.pop(0)()
                    nf -= 1

        def av_chunk(p, c, rs):
            """y^T accumulation for both heads: py[0:65, t] = [y; denom].
            Emitted in r-chunks so it can serve as scalar-free PE filler."""
            if rs[0] == -1:  # prefix: opens the accumulation group. The
                # two K=4 matmuls are row-tiled (strips 0/32) -> concurrent.
                for s in range(2):
                    py = pyp.tile([128, W], F32, tag="py",
                                  name=f"py{p}_{s}_{c}")
                    pys[(p, s, c)] = py
                    nc.tensor.matmul(py[:, :],
                                     vcp[32 * s:32 * s + 4, 2 * p + s, :],
                                     etps[(p, c)][32 * s:32 * s + 4, :],
                                     start=True, stop=False,
                                     tile_position=(32 * s, 0))
                return
            last = 4 * c + 3
            for r in rs:
                tstart = max(0, 128 * r - W * c)
                for s in range(2):
                    nc.tensor.matmul(pys[(p, s, c)][:, tstart:W],
                                     vt[r][:, 2 * p + s, :],
                                     ets[(p, c, r)][:, s, tstart:W],
                                     start=False, stop=(r == last))

        def av_fillers(p, c):
            nr = 4 * c + 4
            return [lambda rs=rs: av_chunk(p, c, rs)
                    for rs in [(-1,)] + [(r,) for r in range(nr)]]

        def av(p, c):
            for f in av_fillers(p, c):
                f()

        def norm_pre(p, c):
            """Denominator reciprocal for window c: head s's denom sits on
            py row 64+s, so after the (required anyway) psum->sbuf copy a
            single-lane DVE reciprocal (SBUF-src 2x mode, ~330ns) writes
            the f16 rrow pair directly -- no DRAM roundtrip, no Sync DMA
            descriptors on the critical path."""
            sbs = []
            dstage = dram.tile([2, W], F32, tag="dst", name=f"dst{p}_{c}")
            for s in range(2):
                py = pys[(p, s, c)]
                sb = sbp.tile([65, W], F32, tag="sb", name=f"sb{p}_{s}_{c}")
                nc.vector.tensor_copy(sb[:], py[0:65, :])
                nc.sync.dma_start(dstage[s:s + 1, :], sb[64:65, :])
                sbs.append(sb)
            dT = drp.tile([64, 16], F32, tag="dT", name=f"dT{p}_{c}")
            nc.sync.dma_start(
                dT[:], dstage[:].rearrange("r (q f) -> (r q) f", f=16))
            rT = drp.tile([64, 16], F32, tag="rT", name=f"rT{p}_{c}")
            nc.vector.reciprocal(rT[:], dT[:])
            rT16 = drp.tile([64, 16], F16, tag="rT16", name=f"rF{p}_{c}")
            with nc.allow_low_precision(reason="softmax denom recip f16"):
                nc.vector.tensor_copy(rT16[:], rT[:])
            d2 = dram.tile([2, W], F16, tag="d2", name=f"d2{p}_{c}")
            nc.sync.dma_start(
                d2[:].rearrange("r (q f) -> (r q) f", f=16), rT16[:])
            rrow = rwp.tile([66, W], F16, tag="rrow", name=f"rw{p}_{c}")
            nc.sync.dma_start(rrow[64:66, :], d2[:])
            return sbs, rrow

        def norm_mul(p, c, sbs, rrow):
            """Broadcast 1/D of BOTH heads in one K=2 f16 matmul (rows
            0:64 head 0, rows 64:128 head 1) and multiply into yT."""
            pb = ps.tile([128, W], F32, tag="ps", name=f"pb{p}_{c}")
            nc.tensor.matmul(pb[:], ones1[64:66, :], rrow[64:66, :],
                             start=True, stop=True)
            for s in range(2):
                nc.vector.tensor_mul(yT[p][64 * s:64 * s + 64,
                                           W * c:W * c + W],
                                     sbs[s][0:64, :],
                                     pb[64 * s:64 * s + 64, :])

        def norm(p, c):
            norm_mul(p, c, *norm_pre(p, c))

        def outproj(mts, kps=range(NPAIR)):
            kps = list(kps)
            for mt in mts:
                osb = op.tile([128, C], F16, tag="osb", name=f"osb{mt}")
                for n0, nsz in ((0, 512), (512, 256)):
                    po = ps.tile([128, 512], F32, tag="ps", name=f"po{mt}_{n0}")
                    for kp in kps:
                        nc.tensor.matmul(po[:, :nsz],
                                         yT[kp][:, 128 * mt:128 * mt + 128],
                                         wp[:, kp, n0:n0 + nsz],
                                         start=(kp == kps[0]),
                                         stop=(kp == kps[-1]))
                    nc.vector.tensor_add(osb[:, n0:n0 + nsz], po[:, :nsz],
                                         bp[:, n0:n0 + nsz])
                nc.sync.dma_start(out_d[128 * mt:128 * mt + 128, :], osb[:])

        spill = [None] * TCH

        def outproj_partial(mt):
            """Pairs 0..4 of chunk mt, spilled (+bias) to an SBUF f32 tile.
            Pure PE+DVE work with no scalar deps -> endgame scores filler."""
            sp = pers.tile([128, C], F32, tag=f"sp{mt}", name=f"sp{mt}")
            spill[mt] = sp
            for n0, nsz in ((0, 512), (512, 256)):
                po = ps.tile([128, 512], F32, tag="ps", name=f"poP{mt}_{n0}")
                for kp in range(NPAIR - 1):
                    nc.tensor.matmul(po[:, :nsz],
                                     yT[kp][:, 128 * mt:128 * mt + 128],
                                     wp[:, kp, n0:n0 + nsz],
                                     start=(kp == 0), stop=(kp == NPAIR - 2))
                nc.vector.tensor_add(sp[:, n0:n0 + nsz], po[:, :nsz],
                                     bp[:, n0:n0 + nsz])

        def outproj_last(mt):
            """Pair-5 contribution + spilled partial -> osb -> DMA out.
            Uses a 2-bank ps2 tile (free once scores end) so two chunks
            stay in flight and the adds overlap the next chunk's matmuls."""
            osb = op.tile([128, C], F16, tag="osb", name=f"osb{mt}")
            po2 = ps2.tile([128, 2, 512], F32, tag="ps2", name=f"poL{mt}")
            for j, (n0, nsz) in enumerate(((0, 512), (512, 256))):
                nc.tensor.matmul(po2[:, j, :nsz],
                                 yT[NPAIR - 1][:, 128 * mt:128 * mt + 128],
                                 wp[:, NPAIR - 1, n0:n0 + nsz],
                                 start=True, stop=True)
            for j, (n0, nsz) in enumerate(((0, 512), (512, 256))):
                nc.vector.tensor_add(osb[:, n0:n0 + nsz], po2[:, j, :nsz],
                                     spill[mt][:, n0:n0 + nsz])
            nc.sync.dma_start(out_d[128 * mt:128 * mt + 128, :], osb[:])

        # ---- emission schedule ----
        # Per unit, merged exp ((2(W-j0)+352)/1.2 ns) outpaces the
        # row-tiled score matmul pair, so every scores window carries
        # scalar-independent PE fillers between its units: qkproj(p+1)
        # in window (p,1), av(p,1) in window (p+1,0), outproj partials
        # (pairs 0..4, spilled to SBUF) in the last pair's windows.
        # norm_mul(p, c) trails by a window so its ones-matmul never
        # waits on the denominator's DRAM roundtrip.
        pl = NPAIR - 1
        qkproj(0)
        for mt in range(4):
            vproj(mt)
        scores_w(0, 0, fillers=[lambda mt=mt: vproj(mt)
                                for mt in range(4, TCH)])
        pending = None
        for p in range(NPAIR):
            av(p, 0)
            n0state = norm_pre(p, 0)
            if p < pl:
                # only the window-0 qkproj groups are needed by
                # scores_w(p+1, 0); the window-1 groups are emitted after
                # it, covering that window's exp/psum drain ahead of
                # av(p+1, 0).
                f = [lambda g=g: qkproj(p + 1, groups=(g,))
                     for g in ((0, 0), (1, 0))]
            else:
                f = [lambda mt=mt: outproj_partial(mt) for mt in range(0, 4)]
            if pending is not None:
                # norm_mul(p-1, 1) fires ~mid-window: the denominator's
                # DMA roundtrip (launched before av(p, 0)) is long back.
                f.insert(1 if p < pl else 2, lambda nm=pending: norm_mul(*nm))
                pending = None
            scores_w(p, 1, fillers=f)
            norm_mul(p, 0, *n0state)
            if p < pl:
                scores_w(p + 1, 0, fillers=av_fillers(p, 1))
                qkproj(p + 1, groups=((0, 1), (1, 1)))
                pending = (p, 1) + tuple(norm_pre(p, 1))
            else:
                av(pl, 1)
                n1state = norm_pre(pl, 1)
                for mt in range(4, TCH):
                    outproj_partial(mt)
                # chunks 0..3 only need yT[5] window 0 (norm_mul(pl, 0)):
                # their finish covers the final denominator roundtrip.
                for mt in range(0, 4):
                    outproj_last(mt)
                # final window: per-128-chunk yT muls so each chunk's
                # pair-5 finish starts right after ITS mul instead of
                # the whole window's (fine-grained PE<->DVE pipeline).
                sbs, rrow = n1state
                pb = ps.tile([128, W], F32, tag="ps", name="pbL")
                nc.tensor.matmul(pb[:], ones1[64:66, :], rrow[64:66, :],
                                 start=True, stop=True)
                for mt in range(4, TCH):
                    c0 = 128 * (mt - 4)
                    for s in range(2):
                        nc.vector.tensor_mul(
                            yT[pl][64 * s:64 * s + 64,
                                   128 * mt:128 * mt + 128],
                            sbs[s][0:64, c0:c0 + 128],
                            pb[64 * s:64 * s + 64, c0:c0 + 128])
                    outproj_last(mt)

    nc.finalize()
    return nc


def _prep_inputs(x, kv_cvec, w_attn, b_attn, w_proj, b_proj):
    x = np.asarray(x, np.float32)
    kv_cvec = np.asarray(kv_cvec, np.float32)
    w_attn = np.asarray(w_attn, np.float32)
    b_attn = np.asarray(b_attn, np.float32)
    w_proj = np.asarray(w_proj, np.float32)
    b_proj = np.asarray(b_proj, np.float32)

    import ml_dtypes

    def to_fp8(a):  # TRN e4 matches OCP e4m3fn for |v| <= 240
        return np.ascontiguousarray(
            np.clip(a, -240.0, 240.0).astype(ml_dtypes.float8_e4m3fn))

    def chunk_major(w):  # [C, N] -> [128, KC, N]
        return np.ascontiguousarray(
            w.reshape(KC, 128, w.shape[1]).transpose(1, 0, 2))

    def dchunk(w):  # [C, ...] -> [128, KC2, 2, ...] DoubleRow fold
        return w.reshape(KC2, 2, 128, -1).transpose(2, 0, 1, 3)

    shared = {
        "wv": chunk_major(w_attn[:, 2 * C:]).astype(np.float16),
        "wp": chunk_major(w_proj).astype(np.float16),
        "b_qk": np.ascontiguousarray(b_attn[:2 * C].reshape(12, 128).T) * WS,
        "bv_bc": np.ascontiguousarray(
            np.broadcast_to(b_attn[2 * C:], (128, C))) * WS,
        "bp_bc": np.ascontiguousarray(np.broadcast_to(b_proj, (128, C))),
        "tri": (np.arange(128)[:, None] <= np.arange(128)[None, :]
                ).astype(np.float16),
        "ones2": np.kron(np.eye(2), np.ones((1, 64))).astype(np.float16),
    }
    for p in range(NPAIR):
        wqp = np.stack([w_attn[:, 128 * p:128 * p + 128],
                        w_attn[:, C + 128 * p:C + 128 * p + 128]], axis=1)
        shared[f"wq{p}"] = chunk_major(
            wqp.reshape(C, 256)).reshape(128, KC, 2, 128).astype(np.float16)

    in_maps = []
    for b in range(N_CORES):
        kc = kv_cvec[b][:, :C].reshape(PFX, H, D)      # [j, h, d]
        vc = kv_cvec[b][:, C:].reshape(PFX, H, D)
        ktc4 = np.zeros((128, NPAIR, 4), np.float32)
        for s in range(2):
            # [d, p, j] block for head 2p+s at partitions 64s:64s+64
            ktc4[64 * s:64 * s + 64, :, :] = \
                kc[:, s::2, :].transpose(2, 1, 0) * WS
        vcp = np.zeros((64, H, 128), np.float32)
        for s0 in (0, 32):
            vcp[s0:s0 + 4, :, :64] = vc * WS
            vcp[s0:s0 + 4, :, 64] = 1.0
        m = dict(shared)
        m["xt"] = np.ascontiguousarray(
            x[b].T.reshape(KC, 128, T).transpose(1, 0, 2)).astype(np.float16)
        m["kTc4"] = ktc4.astype(np.float16)
        m["vcP"] = vcp.astype(np.float16)
        in_maps.append(m)
    return in_maps


_NC_CACHE = {}


def run_hw(trace=False, **inputs):
    """Build+compile+run on 8 NeuronCores; returns (out [8,1024,768], results)."""
    if "nc" not in _NC_CACHE:
        _NC_CACHE["nc"] = _build()
    nc = _NC_CACHE["nc"]
    in_maps = _prep_inputs(**inputs)
    res = run_bass_kernel_spmd(nc, in_maps, list(range(N_CORES)), trace=trace)
    out = np.stack([res.results[b]["out"].astype(np.float32)
                    for b in range(N_CORES)])
    return out, res


def kernel(**inputs):
    out, _ = run_hw(trace=False, **inputs)
    return out



# revision 54
# speedup vs baseline: 1.0200x; 1.0200x over previous
"""Causal self-attention (12 heads, T=1024, C=768, prefix P=4) on 8 TRN2 cores.

Sharding: data-parallel over batch B=8 -> one batch element per NeuronCore.
No collectives. Weights are replicated to every core.

Per-core kernel (fp16 operands, fp32 psum accumulation; fp8 was tried
and rejected -- e4m3 quantization of any of x/wqk/wv alone costs >=1e-2
rel err vs the 2e-2 gate):
  qkv projection from chunk-major tiles: qT pair tile [128, T] (head 2p
    rows 0:64, head 2p+1 rows 64:128), kh pair tile [128, T] (same row
    split), v [128, H, 128] per t-chunk (col 64 = 1.0 -> softmax denom).
  scores (r, window c): the two heads' K=64 matmuls are ROW-TILED
    (tile_position (0,0)/(64,0)) so they run concurrently on the two
    half-arrays, writing the two banks of one 2-bank psum tile; ONE
    merged exp covers both heads ((N+352)/1.2ns amortization). Diagonal
    band tiles get one broadcast triangular-mask multiply; fully-masked
    columns are never computed.
  prefix scores: two row-tiled K=64 matmuls (out rows 0:4 / 32:36) ->
    pp -> two [4, W] exps (only written rows -- reading unwritten psum
    rows is a race); the row-tiled K=4 AV-prefix matmuls OPEN each py
    accumulation group since their exp is ready first.
  AV: py[0:65, t] = [y(64 dims); denom] accumulated over kv chunks.
  norm: DVE copies py -> sb; the two denom rows bounce via DRAM into a
    [64, 16] layout for a lane-parallel reciprocal (DVE reciprocal is
    ~8 cyc/elem/lane -- a [2, W] row recip costs 3.3us, lane-parallel
    costs 0.24us), return as an f16 [2, W] row pair, ONE K=2 f16 matmul
    broadcasts both heads' 1/D, DVE multiplies into the f16 yT tiles.
  out = yT.T @ w_proj + b_proj -> [T, 768] f16 DMA out.

Schedule: merged exp per unit still outpaces the row-tiled matmul pair,
so every scores window interleaves scalar-independent PE fillers between
its units: qkproj(p+1) groups in window (p,1), av(p,1) chunks in window
(p+1,0), and pairs-0..4 output-projection partials (spilled +bias to
SBUF f32) in the last pair's windows. norm_mul trails its norm_pre by a
full window so the ones-matmul never waits on the denominator roundtrip;
the tail is just pair-5 outproj matmuls + fused spill adds.
"""

import numpy as np
from contextlib import ExitStack

import concourse.bass as bass
import concourse.mybir as mybir
import concourse.tile as tile
from concourse import bacc
from concourse.bass_utils import run_bass_kernel_spmd

F32 = mybir.dt.float32
F32R = mybir.dt.float32r
F16 = mybir.dt.float16
N_CORES = 8
T, C, H, D, PFX = 1024, 768, 12, 64, 4
NPAIR = H // 2          # 6 head pairs
KC = C // 128           # 6 contraction chunks
W = 512                 # T window for scores
NW = T // W             # 2 windows
TCH = T // 128          # 8 T chunks
EXP = mybir.ActivationFunctionType.Exp
COPY = mybir.ActivationFunctionType.Copy
F8 = mybir.dt.float8e4
DR = mybir.MatmulPerfMode.DoubleRow
KC2 = KC // 2            # 3 double-chunks of 256 for fp8 DoubleRow
WS = 1.0                 # (fp8 path removed: quantization error ~1.4e-2)
SCALE = 1.0 / np.sqrt(D)
SCALE_E = SCALE / (WS * WS)   # exp scale absorbs q'=WS*q, k'=WS*k


def _build():
    nc = bacc.Bacc("TRN2", target_bir_lowering=False, debug=False,
                   num_devices=N_CORES)
    xt_d = nc.declare_dram_parameter("xt", [128, KC, T], F16, isOutput=False)
    wq_d = [nc.declare_dram_parameter(f"wq{p}", [128, KC, 2, 128], F16,
                                      isOutput=False) for p in range(NPAIR)]
    wv_d = nc.declare_dram_parameter("wv", [128, KC, C], F16, isOutput=False)
    wp_d = nc.declare_dram_parameter("wp", [128, KC, C], F16, isOutput=False)
    bqk_d = nc.declare_dram_parameter("b_qk", [128, 12], F32, isOutput=False)
    bv_d = nc.declare_dram_parameter("bv_bc", [128, C], F32, isOutput=False)
    bp_d = nc.declare_dram_parameter("bp_bc", [128, C], F32, isOutput=False)
    ktc_d = nc.declare_dram_parameter("kTc4", [128, NPAIR, 4], F16,
                                      isOutput=False)
    vcp_d = nc.declare_dram_parameter("vcP", [64, H, 128], F16, isOutput=False)
    tri_d = nc.declare_dram_parameter("tri", [128, 128], F16, isOutput=False)
    ones_d = nc.declare_dram_parameter("ones2", [2, 128], F16, isOutput=False)
    out_d = nc.declare_dram_parameter("out", [T, C], F16, isOutput=True)

    with tile.TileContext(nc) as tc, ExitStack() as ctx:
        pers = ctx.enter_context(tc.tile_pool(name="pers", bufs=1))
        wqp = ctx.enter_context(tc.tile_pool(name="wqp", bufs=6))
        qkp = ctx.enter_context(tc.tile_pool(name="qkp", bufs=2))
        khp = ctx.enter_context(tc.tile_pool(name="khp", bufs=2))
        ep = ctx.enter_context(tc.tile_pool(name="ep", bufs=13))
        epp = ctx.enter_context(tc.tile_pool(name="epp", bufs=6))
        sbp = ctx.enter_context(tc.tile_pool(name="sbp", bufs=6))
        rwp = ctx.enter_context(tc.tile_pool(name="rwp", bufs=2))
        drp = ctx.enter_context(tc.tile_pool(name="drp", bufs=2))
        dram = ctx.enter_context(tc.tile_pool(name="dram", bufs=4,
                                              space="DRAM"))
        op = ctx.enter_context(tc.tile_pool(name="op", bufs=4))
        ps = ctx.enter_context(tc.tile_pool(name="ps", bufs=2, space="PSUM"))
        ps2 = ctx.enter_context(tc.tile_pool(name="ps2", bufs=2, space="PSUM"))
        pyp = ctx.enter_context(tc.tile_pool(name="pyp", bufs=2, space="PSUM"))

        # ---- PE warmup: HAM needs ~3.4us of activity to unthrottle ------
        wtile = pers.tile([128, W], F16, tag="wtile")
        nc.vector.memset(wtile[:], 0.0)
        pwarm = ps.tile([128, 512], F32, tag="ps", name="pwarm")
        for i in range(12):
            nc.tensor.matmul(pwarm[:], wtile[:, 0:128], wtile[:],
                             start=True, stop=True)

        # ---- persistent loads, priority order ---------------------------
        wq = [None] * NPAIR
        wq[0] = wqp.tile([128, KC, 2, 128], F16, tag="wq", name="wq0")
        nc.sync.dma_start(wq[0][:], wq_d[0][:])
        xt = pers.tile([128, KC, T], F16, tag="xt")
        for k in range(KC):  # per-chunk so the first qkproj matmuls start
            nc.sync.dma_start(xt[:, k, :], xt_d[:, k, :])
        bqk = pers.tile([128, 12], F32, tag="bqk")
        nc.sync.dma_start(bqk[:], bqk_d[:])
        wv = pers.tile([128, KC, C], F16, tag="wv")
        nc.sync.dma_start(wv[:], wv_d[:])
        bv = pers.tile([128, C], F32, tag="bv")
        nc.sync.dma_start(bv[:], bv_d[:])
        tri = pers.tile([128, 128], F16, tag="tri")
        nc.sync.dma_start(tri[:], tri_d[:])
        ktc = pers.tile([128, NPAIR, 4], F16, tag="ktc")
        nc.sync.dma_start(ktc[:], ktc_d[:])
        vcp = pers.tile([64, H, 128], F16, tag="vcp")
        nc.sync.dma_start(vcp[:], vcp_d[:])
        for p in range(1, NPAIR):
            wq[p] = wqp.tile([128, KC, 2, 128], F16, tag="wq",
                             name=f"wq{p}")
            nc.sync.dma_start(wq[p][:], wq_d[p][:])

        wp = pers.tile([128, KC, C], F16, tag="wp")
        nc.sync.dma_start(wp[:], wp_d[:])
        bp = pers.tile([128, C], F32, tag="bp")
        nc.sync.dma_start(bp[:], bp_d[:])

        ones1 = pers.tile([66, 128], F16, tag="ones1")
        nc.sync.dma_start(ones1[64:66, :], ones_d[:])

        yT = [pers.tile([128, T], F16, tag=f"yT{p}", name=f"yT{p}")
              for p in range(NPAIR)]

        # ---- phases ----
        qk_tiles = {}
        ets = {}
        etps = {}
        pys = {}

        def qkproj(p, groups=((0, 0), (0, 1), (1, 0), (1, 1))):
            if p in qk_tiles:
                qT, kh = qk_tiles[p]
            else:
                qT = qkp.tile([128, T], F16, tag="qT", name=f"qT{p}")
                # pair tile: head 2p's k-features at rows 0:64, head 2p+1's
                # at rows 64:128 (aligned with the q rows in qT).
                kh = khp.tile([128, T], F16, tag="kh", name=f"kh{p}")
                qk_tiles[p] = (qT, kh)
            for half, w in groups:
                pq = ps.tile([128, 512], F32, tag="ps",
                             name=f"pq{p}_{half}_{w}")
                for k in range(KC):
                    nc.tensor.matmul(pq[:], wq[p][:, k, half, :],
                                     xt[:, k, W * w:W * w + W],
                                     start=(k == 0), stop=(k == KC - 1))
                if half == 0:
                    nc.vector.tensor_scalar_add(
                        qT[:, W * w:W * w + W], pq[:], bqk[:, p:p + 1])
                else:
                    nc.vector.tensor_scalar_add(
                        kh[:, W * w:W * w + W], pq[:], bqk[:, 6 + p:7 + p])

        vt = [None] * TCH

        def vproj(mt):
            v_ = pers.tile([128, H, 128], F16, tag=f"v{mt}")
            nc.gpsimd.memset(v_[:, :, 64:65], 1.0)
            nc.gpsimd.memset(v_[:, :, 65:128], 0.0)
            for n0, nsz in ((0, 512), (512, 256)):
                pv = ps.tile([128, 512], F32, tag="ps", name=f"pv{mt}_{n0}")
                for k in range(KC):
                    nc.tensor.matmul(pv[:, :nsz],
                                     xt[:, k, 128 * mt:128 * mt + 128],
                                     wv[:, k, n0:n0 + nsz],
                                     start=(k == 0), stop=(k == KC - 1))
                h0, hn = n0 // 64, nsz // 64
                nc.vector.tensor_add(
                    v_[:, h0:h0 + hn, 0:64],
                    pv[:, :nsz].rearrange("a (h d) -> a h d", d=64),
                    bv[:, n0:n0 + nsz].rearrange("a (h d) -> a h d", d=64))
            vt[mt] = v_

        def scores_w(p, c, fillers=()):
            """Window c of pair p. The two heads' K=64 score matmuls are
            row-tiled (tile_position (0,0)/(64,0)) -> they run CONCURRENTLY
            on the upper/lower halves of the PE array, writing the two
            banks of one 2-bank psum tile; ONE merged exp covers both
            heads ((N+352)/1.2 scalar cost amortizes the 352-cy overhead).
            `fillers` are scalar-independent emission thunks (qkproj
            groups / vproj chunks) interleaved between units so the PE
            never starves while the exp queue drains."""
            fillers = list(fillers)
            qT, kh = qk_tiles[p]
            # prefix first: the two heads' K=64 prefix matmuls are also
            # row-tiled (out rows 0:4 and 32:36); their exp lands early
            # in the scalar queue so the AV-prefix matmuls never stall.
            pp = ps.tile([128, 512], F32, tag="ps", name=f"pp{p}_{c}")
            for s in range(2):
                nc.tensor.matmul(pp[32 * s:32 * s + 4, :],
                                 ktc[64 * s:64 * s + 64, p, :],
                                 qT[64 * s:64 * s + 64,
                                    W * c:W * (c + 1)],
                                 start=True, stop=True,
                                 tile_position=(64 * s, 32 * s))
            ep_ = epp.tile([36, W], F16, tag="etp", name=f"etp{p}_{c}")
            for s in range(2):  # only rows 32s:32s+4 were written
                nc.scalar.activation(ep_[32 * s:32 * s + 4, :],
                                     pp[32 * s:32 * s + 4, :], EXP,
                                     scale=float(SCALE_E))
            etps[(p, c)] = ep_
            nun = 4 * c + 4
            for r in range(nun):
                # both heads' e for (c, r) share one [128, 2, W] tile
                e2 = ep.tile([128, 2, W], F16, tag="et",
                             name=f"et{p}_{c}_{r}")
                ets[(p, c, r)] = e2
                j0 = 128 * r - W * c if r >= 4 * c else 0
                pt = ps2.tile([128, 2, 512], F32, tag="ps2",
                              name=f"pss{p}_{c}_{r}")
                for s in range(2):
                    nc.tensor.matmul(
                        pt[:, s, j0:W],
                        kh[64 * s:64 * s + 64, 128 * r:128 * r + 128],
                        qT[64 * s:64 * s + 64, W * c + j0:W * (c + 1)],
                        start=True, stop=True,
                        tile_position=(64 * s, 0))
                nc.scalar.activation(e2[:, :, j0:W], pt[:, :, j0:W],
                                     EXP, scale=float(SCALE_E))
                if r >= 4 * c:  # one masked multiply covers both heads
                    nc.vector.tensor_mul(
                        e2[:, :, j0:j0 + 128], e2[:, :, j0:j0 + 128],
                        tri[:].unsqueeze(1).broadcast_to((128, 2, 128)))
                # spread fillers across the remaining units
                nf = len(fillers) * (r + 1) // nun
                while fillers and nf:
                    fillers.pop(0)()
                    nf -= 1

        def av_chunk(p, c, rs):
            """y^T accumulation for both heads: py[0:65, t] = [y; denom].
            Emitted in r-chunks so it can serve as scalar-free PE filler."""
            if rs[0] == -1:  # prefix: opens the accumulation group. The
                # two K=4 matmuls are row-tiled (strips 0/32) -> concurrent.
                for s in range(2):
                    py = pyp.tile([128, W], F32, tag="py",
                                  name=f"py{p}_{s}_{c}")
                    pys[(p, s, c)] = py
                    nc.tensor.matmul(py[:, :],
                                     vcp[32 * s:32 * s + 4, 2 * p + s, :],
                                     etps[(p, c)][32 * s:32 * s + 4, :],
                                     start=True, stop=False,
                                     tile_position=(32 * s, 0))
                return
            last = 4 * c + 3
            for r in rs:
                tstart = max(0, 128 * r - W * c)
                for s in range(2):
                    nc.tensor.matmul(pys[(p, s, c)][:, tstart:W],
                                     vt[r][:, 2 * p + s, :],
                                     ets[(p, c, r)][:, s, tstart:W],
                                     start=False, stop=(r == last))

        def av_fillers(p, c):
            nr = 4 * c + 4
            return [lambda rs=rs: av_chunk(p, c, rs)
                    for rs in [(-1,)] + [(r,) for r in range(nr)]]

        def av(p, c):
            for f in av_fillers(p, c):
                f()

        def norm_pre(p, c):
            """Denominator reciprocal for window c: head s's denom sits on
            py row 64+s, so after the (required anyway) psum->sbuf copy a
            single-lane DVE reciprocal (SBUF-src 2x mode, ~330ns) writes
            the f16 rrow pair directly -- no DRAM roundtrip, no Sync DMA
            descriptors on the critical path."""
            sbs = []
            dstage = dram.tile([2, W], F32, tag="dst", name=f"dst{p}_{c}")
            for s in range(2):
                py = pys[(p, s, c)]
                sb = sbp.tile([65, W], F32, tag="sb", name=f"sb{p}_{s}_{c}")
                nc.vector.tensor_copy(sb[:], py[0:65, :])
                nc.sync.dma_start(dstage[s:s + 1, :], sb[64:65, :])
                sbs.append(sb)
            dT = drp.tile([64, 16], F32, tag="dT", name=f"dT{p}_{c}")
            nc.sync.dma_start(
                dT[:], dstage[:].rearrange("r (q f) -> (r q) f", f=16))
            rT = drp.tile([64, 16], F32, tag="rT", name=f"rT{p}_{c}")
            nc.vector.reciprocal(rT[:], dT[:])
            rT16 = drp.tile([64, 16], F16, tag="rT16", name=f"rF{p}_{c}")
            with nc.allow_low_precision(reason="softmax denom recip f16"):
                nc.vector.tensor_copy(rT16[:], rT[:])
            d2 = dram.tile([2, W], F16, tag="d2", name=f"d2{p}_{c}")
            nc.sync.dma_start(
                d2[:].rearrange("r (q f) -> (r q) f", f=16), rT16[:])
            rrow = rwp.tile([66, W], F16, tag="rrow", name=f"rw{p}_{c}")
            nc.sync.dma_start(rrow[64:66, :], d2[:])
            return sbs, rrow

        def norm_mul(p, c, sbs, rrow):
            """Broadcast 1/D of BOTH heads in one K=2 f16 matmul (rows
            0:64 head 0, rows 64:128 head 1) and multiply into yT."""
            pb = ps.tile([128, W], F32, tag="ps", name=f"pb{p}_{c}")
            nc.tensor.matmul(pb[:], ones1[64:66, :], rrow[64:66, :],
                             start=True, stop=True)
            for s in range(2):
                nc.vector.tensor_mul(yT[p][64 * s:64 * s + 64,
                                           W * c:W * c + W],
                                     sbs[s][0:64, :],
                                     pb[64 * s:64 * s + 64, :])

        def norm(p, c):
            norm_mul(p, c, *norm_pre(p, c))

        def outproj(mts, kps=range(NPAIR)):
            kps = list(kps)
            for mt in mts:
                osb = op.tile([128, C], F16, tag="osb", name=f"osb{mt}")
                for n0, nsz in ((0, 512), (512, 256)):
                    po = ps.tile([128, 512], F32, tag="ps", name=f"po{mt}_{n0}")
                    for kp in kps:
                        nc.tensor.matmul(po[:, :nsz],
                                         yT[kp][:, 128 * mt:128 * mt + 128],
                                         wp[:, kp, n0:n0 + nsz],
                                         start=(kp == kps[0]),
                                         stop=(kp == kps[-1]))
                    nc.vector.tensor_add(osb[:, n0:n0 + nsz], po[:, :nsz],
                                         bp[:, n0:n0 + nsz])
                nc.sync.dma_start(out_d[128 * mt:128 * mt + 128, :], osb[:])

        spill = [None] * TCH

        def outproj_partial(mt):
            """Pairs 0..4 of chunk mt, spilled (+bias) to an SBUF f32 tile.
            Pure PE+DVE work with no scalar deps -> endgame scores filler."""
            sp = pers.tile([128, C], F32, tag=f"sp{mt}", name=f"sp{mt}")
            spill[mt] = sp
            for n0, nsz in ((0, 512), (512, 256)):
                po = ps.tile([128, 512], F32, tag="ps", name=f"poP{mt}_{n0}")
                for kp in range(NPAIR - 1):
                    nc.tensor.matmul(po[:, :nsz],
                                     yT[kp][:, 128 * mt:128 * mt + 128],
                                     wp[:, kp, n0:n0 + nsz],
                                     start=(kp == 0), stop=(kp == NPAIR - 2))
                nc.vector.tensor_add(sp[:, n0:n0 + nsz], po[:, :nsz],
                                     bp[:, n0:n0 + nsz])

        def outproj_last(mt):
            """Pair-5 contribution + spilled partial -> osb -> DMA out.
            Uses a 2-bank ps2 tile (free once scores end) so two chunks
            stay in flight and the adds overlap the next chunk's matmuls."""
            osb = op.tile([128, C], F16, tag="osb", name=f"osb{mt}")
            po2 = ps2.tile([128, 2, 512], F32, tag="ps2", name=f"poL{mt}")
            for j, (n0, nsz) in enumerate(((0, 512), (512, 256))):
                nc.tensor.matmul(po2[:, j, :nsz],
                                 yT[NPAIR - 1][:, 128 * mt:128 * mt + 128],
                                 wp[:, NPAIR - 1, n0:n0 + nsz],
                                 start=True, stop=True)
            for j, (n0, nsz) in enumerate(((0, 512), (512, 256))):
                nc.vector.tensor_add(osb[:, n0:n0 + nsz], po2[:, j, :nsz],
                                     spill[mt][:, n0:n0 + nsz])
            nc.sync.dma_start(out_d[128 * mt:128 * mt + 128, :], osb[:])

        # ---- emission schedule ----
        # Per unit, merged exp ((2(W-j0)+352)/1.2 ns) outpaces the
        # row-tiled score matmul pair, so every scores window carries
        # scalar-independent PE fillers between its units: qkproj(p+1)
        # in window (p,1), av(p,1) in window (p+1,0), outproj partials
        # (pairs 0..4, spilled to SBUF) in the last pair's windows.
        # norm_mul(p, c) trails by a window so its ones-matmul never
        # waits on the denominator's DRAM roundtrip.
        pl = NPAIR - 1
        qkproj(0)
        for mt in range(4):
            vproj(mt)
        scores_w(0, 0, fillers=[lambda mt=mt: vproj(mt)
                                for mt in range(4, TCH)])
        pending = None
        for p in range(NPAIR):
            av(p, 0)
            n0state = norm_pre(p, 0)
            if p < pl:
                f = [lambda g=g: qkproj(p + 1, groups=(g,))
                     for g in ((0, 0), (0, 1), (1, 0), (1, 1))]
            else:
                f = [lambda mt=mt: outproj_partial(mt) for mt in range(0, 4)]
            if pending is not None:
                # norm_mul(p-1, 1) fires ~mid-window: the denominator's
                # DMA roundtrip (launched before av(p, 0)) is long back.
                f.insert(2, lambda nm=pending: norm_mul(*nm))
                pending = None
            scores_w(p, 1, fillers=f)
            norm_mul(p, 0, *n0state)
            if p < pl:
                scores_w(p + 1, 0, fillers=av_fillers(p, 1))
                pending = (p, 1) + tuple(norm_pre(p, 1))
            else:
                av(pl, 1)
                n1state = norm_pre(pl, 1)
                for mt in range(4, TCH):
                    outproj_partial(mt)
                # chunks 0..3 only need yT[5] window 0 (norm_mul(pl, 0)):
                # their finish covers the final denominator roundtrip.
                for mt in range(0, 4):
                    outproj_last(mt)
                # final window: per-128-chunk yT muls so each chunk's
                # pair-5 finish starts right after ITS mul instead of
                # the whole window's (fine-grained PE<->DVE pipeline).
                sbs, rrow = n1state
                pb = ps.tile([128, W], F32, tag="ps", name="pbL")
                nc.tensor.matmul(pb[:], ones1[64:66, :], rrow[64:66, :],
                                 start=True, stop=True)
                for mt in range(4, TCH):
                    c0 = 128 * (mt - 4)
                    for s in range(2):
                        nc.vector.tensor_mul(
                            yT[pl][64 * s:64 * s + 64,
                                   128 * mt:128 * mt + 128],
                            sbs[s][0:64, c0:c0 + 128],
                            pb[64 * s:64 * s + 64, c0:c0 + 128])
                    outproj_last(mt)

    nc.finalize()
    return nc


def _prep_inputs(x, kv_cvec, w_attn, b_attn, w_proj, b_proj):
    x = np.asarray(x, np.float32)
    kv_cvec = np.asarray(kv_cvec, np.float32)
    w_attn = np.asarray(w_attn, np.float32)
    b_attn = np.asarray(b_attn, np.float32)
    w_proj = np.asarray(w_proj, np.float32)
    b_proj = np.asarray(b_proj, np.float32)

    import ml_dtypes

    def to_fp8(a):  # TRN e4 matches OCP e4m3fn for |v| <= 240
        return np.ascontiguousarray(
            np.clip(a, -240.0, 240.0).astype(ml_dtypes.float8_e4m3fn))

    def chunk_major(w):  # [C, N] -> [128, KC, N]
        return np.ascontiguousarray(
            w.reshape(KC, 128, w.shape[1]).transpose(1, 0, 2))

    def dchunk(w):  # [C, ...] -> [128, KC2, 2, ...] DoubleRow fold
        return w.reshape(KC2, 2, 128, -1).transpose(2, 0, 1, 3)

    shared = {
        "wv": chunk_major(w_attn[:, 2 * C:]).astype(np.float16),
        "wp": chunk_major(w_proj).astype(np.float16),
        "b_qk": np.ascontiguousarray(b_attn[:2 * C].reshape(12, 128).T) * WS,
        "bv_bc": np.ascontiguousarray(
            np.broadcast_to(b_attn[2 * C:], (128, C))) * WS,
        "bp_bc": np.ascontiguousarray(np.broadcast_to(b_proj, (128, C))),
        "tri": (np.arange(128)[:, None] <= np.arange(128)[None, :]
                ).astype(np.float16),
        "ones2": np.kron(np.eye(2), np.ones((1, 64))).astype(np.float16),
    }
    for p in range(NPAIR):
        wqp = np.stack([w_attn[:, 128 * p:128 * p + 128],
                        w_attn[:, C + 128 * p:C + 128 * p + 128]], axis=1)
        shared[f"wq{p}"] = chunk_major(
            wqp.reshape(C, 256)).reshape(128, KC, 2, 128).astype(np.float16)

    in_maps = []
    for b in range(N_CORES):
        kc = kv_cvec[b][:, :C].reshape(PFX, H, D)      # [j, h, d]
        vc = kv_cvec[b][:, C:].reshape(PFX, H, D)
        ktc4 = np.zeros((128, NPAIR, 4), np.float32)
        for s in range(2):
            # [d, p, j] block for head 2p+s at partitions 64s:64s+64
            ktc4[64 * s:64 * s + 64, :, :] = \
                kc[:, s::2, :].transpose(2, 1, 0) * WS
        vcp = np.zeros((64, H, 128), np.float32)
        for s0 in (0, 32):
            vcp[s0:s0 + 4, :, :64] = vc * WS
            vcp[s0:s0 + 4, :, 64] = 1.0
        m = dict(shared)
        m["xt"] = np.ascontiguousarray(
            x[b].T.reshape(KC, 128, T).transpose(1, 0, 2)).astype(np.float16)
        m["kTc4"] = ktc4.astype(np.float16)
        m["vcP"] = vcp.astype(np.float16)
        in_maps.append(m)
    return in_maps


_NC_CACHE = {}


def run_hw(trace=False, **inputs):
    """Build+compile+run on 8 NeuronCores; returns (out [8,1024,768], results)."""
    if "nc" not in _NC_CACHE:
        _NC_CACHE["nc"] = _build()
    nc = _NC_CACHE["nc"]
    in_maps = _prep_inputs(**inputs)
    res = run_bass_kernel_spmd(nc, in_maps, list(range(N_CORES)), trace=trace)
    out = np.stack([res.results[b]["out"].astype(np.float32)
                    for b in range(N_CORES)])
    return out, res


def kernel(**inputs):
    out, _ = run_hw(trace=False, **inputs)
    return out

